# revision 9
# baseline (speedup 1.0000x reference)
"""DimeNet++ Trainium2 kernel (8 NeuronCores, graph-parallel).

Self-contained: takes full inputs, shards internally, runs one SPMD Bass
program on 8 cores, gathers the full output.

Layout strategy (per core c of 8):
 - Edges sorted by idx_i (target node), nodes split into 8 contiguous
   ranges with ~equal edge counts. Each core owns its node range and the
   edges pointing into it. Edge stream padded so every 128-node window
   owns exactly C2 edge tiles of 128 (E_pad = NWN*C2*128).
 - Triplets assigned to the core owning their destination edge (idx_ji),
   sorted by destination edge window (128 edges), padded so every window
   owns exactly C1 chunks of 128 triplet rows (T_pad = NWE*C1*128).
 - Activations kept feature-major ([H=128 partitions, edges]) in SBUF
   across all 4 interaction blocks; x_kj_down is all-gathered (collective)
   across cores each block; triplet gather via indirect DMA; both
   segment-sums (idx_ji and idx_i) are done as one-hot matmuls built from
   iota/is_equal compares, accumulating in PSUM.
"""

import math
import os

import numpy as np

H, IE, BE, NS, NR, OE, OC = 128, 64, 8, 7, 6, 256, 1
NB = 4
NSR = NS * NR  # 42
P = 128


# ---------------------------------------------------------------- host prep
def _ceil_div(a, b):
    return -(-a // b)


def host_prep(x, rbf, sbf, idx_kj, idx_ji, idx_i, num_nodes, ncores):
    import ml_dtypes

    bf = ml_dtypes.bfloat16
    E = x.shape[0]
    T = sbf.shape[0]
    N = int(num_nodes)
    idx_i = np.asarray(idx_i, np.int64)
    idx_kj = np.asarray(idx_kj, np.int64)
    idx_ji = np.asarray(idx_ji, np.int64)

    # ---- core node boundaries, balancing edge counts
    counts_n = np.bincount(idx_i, minlength=N)
    cumn = np.concatenate([[0], np.cumsum(counts_n)])  # cumn[n] = #edges node < n
    bounds = [0]
    for c in range(1, ncores):
        bounds.append(int(np.searchsorted(cumn, c * E / ncores)))
    bounds.append(N)
    n_lo = np.array(bounds[:-1])
    n_hi = np.array(bounds[1:])
    nodes_c = n_hi - n_lo
    NWN = int(max(_ceil_div(int(nc_), P) for nc_ in nodes_c))

    # ---- C2: edge tiles per 128-node window
    C2 = 1
    win_edge_cnt = np.zeros((ncores, NWN), np.int64)
    for c in range(ncores):
        for w in range(NWN):
            a = min(int(n_lo[c]) + P * w, int(n_hi[c]))
            b = min(a + P, int(n_hi[c]))
            cnt = int(cumn[b] - cumn[a])
            win_edge_cnt[c, w] = cnt
            C2 = max(C2, _ceil_div(cnt, P))
    E_pad = NWN * C2 * P
    NWE = NWN * C2

    # ---- edge placement
    eperm = np.argsort(idx_i, kind="stable")  # edges ordered by node
    g2l = np.full(E, -1, np.int64)
    edge_list = np.full((ncores, E_pad), -1, np.int64)
    for c in range(ncores):
        for w in range(NWN):
            a = min(int(n_lo[c]) + P * w, int(n_hi[c]))
            b = min(a + P, int(n_hi[c]))
            lo, hi = int(cumn[a]), int(cumn[b])
            cnt = hi - lo
            base = w * C2 * P
            edge_list[c, base : base + cnt] = eperm[lo:hi]
            g2l[eperm[lo:hi]] = c * E_pad + base + np.arange(cnt)

    x_fm = np.zeros((ncores, H, E_pad), bf)
    x_rm = np.zeros((ncores, E_pad, H), bf)
    rbfT = np.zeros((ncores, NR, E_pad), bf)
    off2 = np.zeros((ncores, P, NWE), np.float16)
    for c in range(ncores):
        el = edge_list[c]
        v = el >= 0
        xr = np.zeros((E_pad, H), np.float32)
        xr[v] = x[el[v]]
        x_rm[c] = xr.astype(bf)
        x_fm[c] = xr.T.astype(bf)
        rr = np.zeros((E_pad, NR), np.float32)
        rr[v] = rbf[el[v]]
        rbfT[c] = rr.T.astype(bf)
        o2 = np.zeros(E_pad, np.float16)
        nw = np.arange(E_pad) // (C2 * P)  # node window of each slot
        o2[v] = (idx_i[el[v]] - (n_lo[c] + P * nw[v])).astype(np.float16)
        off2[c] = o2.reshape(NWE, P).T

    # ---- triplets
    dest_g = g2l[idx_ji]
    src_g = g2l[idx_kj]
    assert dest_g.min() >= 0 and src_g.min() >= 0
    dest_c = dest_g // E_pad
    dest_l = dest_g % E_pad

    # C1: chunks per 128-edge window
    C1 = 1
    per_core = []
    for c in range(ncores):
        m = dest_c == c
        dl = dest_l[m]
        sg = src_g[m]
        rows = np.nonzero(m)[0]
        win = dl >> 7
        order = np.lexsort((sg, win))
        dl, sg, rows, win = dl[order], sg[order], rows[order], win[order]
        wcnt = np.bincount(win, minlength=NWE)
        C1 = max(C1, int(_ceil_div(int(wcnt.max()), P)))
        per_core.append((dl, sg, rows, win, wcnt))

    T_pad = NWE * C1 * P
    NCHUNK = NWE * C1
    sbfT = np.zeros((ncores, NSR, T_pad), bf)
    gidx = np.zeros((ncores, P, NCHUNK), np.int32)
    off1 = np.zeros((ncores, P, NCHUNK), np.float16)
    for c in range(ncores):
        dl, sg, rows, win, wcnt = per_core[c]
        wstart = np.concatenate([[0], np.cumsum(wcnt)])[:-1]
        pos = win * (C1 * P) + (np.arange(len(dl)) - wstart[win])
        sp = np.zeros((T_pad, NSR), np.float32)
        sp[pos] = sbf[rows]
        sbfT[c] = sp.T.astype(bf)
        gi = np.zeros(T_pad, np.int32)
        gi[pos] = sg.astype(np.int32)
        gidx[c] = gi.reshape(NCHUNK, P).T
        o1 = np.zeros(T_pad, np.float16)
        o1[pos] = (dl & 127).astype(np.float16)
        off1[c] = o1.reshape(NCHUNK, P).T

    dims = dict(
        E_pad=E_pad, NWE=NWE, NWN=NWN, C1=C1, C2=C2, T_pad=T_pad,
        NCHUNK=NCHUNK, ncores=ncores, n_lo=n_lo.tolist(), nodes_c=nodes_c.tolist(),
    )
    data = dict(x_fm=x_fm, x_rm=x_rm, rbfT=rbfT, off2=off2, sbfT=sbfT,
                gidx=gidx, off1=off1)
    return dims, data


def prep_weights(ws):
    """ws: dict of the weight arrays from setup_inputs (numpy float32)."""
    import ml_dtypes

    bf = ml_dtypes.bfloat16
    f32 = np.float32
    out = {}
    out["Wkj"] = ws["Wkj"].astype(bf)                    # [NB,128,128]
    out["Wji"] = ws["Wji"].astype(bf)
    out["Wlin"] = ws["Wlin"].astype(bf)
    out["Wres"] = ws["Wres"].astype(bf)                  # [NB,3,2,128,128]
    out["Wdown"] = ws["Wdown"].astype(bf)                # [NB,128,64]
    out["Wup"] = ws["Wup"].astype(bf)                    # [NB,64,128]
    out["W12r"] = np.einsum("bij,bjk->bik", ws["Wrbf1"], ws["Wrbf2"]).astype(bf)   # [NB,6,128]
    out["W12s"] = np.einsum("bij,bjk->bik", ws["Wsbf1"], ws["Wsbf2"]).astype(bf)   # [NB,42,64]
    out["Worbf"] = ws["Worbf"].astype(bf)                # [NB+1,6,128]
    out["Woup"] = ws["Woup"].astype(bf)                  # [NB+1,128,256]
    # Wouts [NB+1,3,256,256] -> [NB+1,3,2(in half),128,256]
    out["Wouts"] = ws["Wouts"].reshape(NB + 1, 3, 2, 128, OE).astype(bf)
    out["Wo"] = ws["Wo"].reshape(NB + 1, 2, 128, OC).astype(bf)   # [NB+1,2,128,1]
    out["bkj"] = ws["bkj"].reshape(NB, H, 1).astype(f32)
    out["bji"] = ws["bji"].reshape(NB, H, 1).astype(f32)
    out["blin"] = ws["blin"].reshape(NB, H, 1).astype(f32)
    out["bres"] = ws["bres"].reshape(NB, 3, 2, H, 1).astype(f32)
    out["boup"] = ws["boup"].reshape(NB + 1, OE, 1).astype(f32)   # [NB+1,256,1]
    out["bouts"] = ws["bouts"].reshape(NB + 1, 3, OE, 1).astype(f32)
    return out


# ---------------------------------------------------------------- bass build
def build_kernel(nc, dims, sim_safe=False):
    import concourse.bass as bass
    import concourse.tile as tile
    from concourse import mybir

    f32 = mybir.dt.float32
    bf16 = mybir.dt.bfloat16
    fp16 = mybir.dt.float16
    i32 = mybir.dt.int32
    AF = mybir.ActivationFunctionType
    SILU = AF.Sigmoid if sim_safe else AF.Silu

    E_pad = dims["E_pad"]; NWE = dims["NWE"]; NWN = dims["NWN"]
    C1 = dims["C1"]; C2 = dims["C2"]; T_pad = dims["T_pad"]
    NCHUNK = dims["NCHUNK"]; ncores = dims["ncores"]

    # ---------------- dram I/O
    d_xfm = nc.dram_tensor("x_fm", [H, E_pad], bf16, kind="ExternalInput").ap()
    d_xrm = nc.dram_tensor("x_rm", [E_pad, H], bf16, kind="ExternalInput").ap()
    d_rbfT = nc.dram_tensor("rbfT", [NR, E_pad], bf16, kind="ExternalInput").ap()
    d_sbfT = nc.dram_tensor("sbfT", [NSR, T_pad], bf16, kind="ExternalInput").ap()
    d_gidx = nc.dram_tensor("gidx", [P, NCHUNK], i32, kind="ExternalInput").ap()
    d_off1 = nc.dram_tensor("off1", [P, NCHUNK], fp16, kind="ExternalInput").ap()
    d_off2 = nc.dram_tensor("off2", [P, NWE], fp16, kind="ExternalInput").ap()

    wd = {}
    for nm, shp, dt in [
        ("Wkj", [NB, H, H], bf16), ("Wji", [NB, H, H], bf16),
        ("Wlin", [NB, H, H], bf16), ("Wres", [NB, 3, 2, H, H], bf16),
        ("Wdown", [NB, H, IE], bf16), ("Wup", [NB, IE, H], bf16),
        ("W12r", [NB, NR, H], bf16), ("W12s", [NB, NSR, IE], bf16),
        ("Worbf", [NB + 1, NR, H], bf16), ("Woup", [NB + 1, H, OE], bf16),
        ("Wouts", [NB + 1, 3, 2, 128, OE], bf16), ("Wo", [NB + 1, 2, 128, OC], bf16),
        ("bkj", [NB, H, 1], f32), ("bji", [NB, H, 1], f32),
        ("blin", [NB, H, 1], f32), ("bres", [NB, 3, 2, H, 1], f32),
        ("boup", [NB + 1, OE, 1], f32), ("bouts", [NB + 1, 3, OE, 1], f32),
    ]:
        wd[nm] = nc.dram_tensor(nm, shp, dt, kind="ExternalInput").ap()

    d_P = nc.dram_tensor("P_out", [1, NWN * P], f32, kind="ExternalOutput").ap()

    ag_in = nc.dram_tensor("ag_in", [E_pad, IE], bf16, kind="Internal").ap()
    ag_out = nc.dram_tensor(
        "ag_out", [ncores * E_pad, IE], bf16, kind="Internal",
        addr_space="Shared" if ncores > 4 else "Local",
    ).ap()

    KB = 16  # chunk batch in triplet phase
    GW1 = 8   # tiles per phase-1 group (1024 edge cols)
    GW3 = 8   # windows per phase-2/3 group

    with tile.TileContext(nc) as tc:
        import contextlib
        ctx = contextlib.ExitStack()
        with ctx:
            per = ctx.enter_context(tc.tile_pool(name="per", bufs=1))
            cst = ctx.enter_context(tc.tile_pool(name="cst", bufs=1))
            sbp = ctx.enter_context(tc.tile_pool(name="sbp", bufs=2))
            hch = ctx.enter_context(tc.tile_pool(name="hch", bufs=2))
            pbig = ctx.enter_context(tc.tile_pool(name="pbig", bufs=3, space="PSUM"))
            pagg = ctx.enter_context(tc.tile_pool(name="pagg", bufs=2, space="PSUM"))

            # ---------------- persistent tiles
            xe = per.tile([H, E_pad], bf16, tag="xe")
            off1_sb = per.tile([P, NCHUNK], fp16, tag="off1")
            gidx_sb = per.tile([P, NCHUNK], i32, tag="gidx")
            off2_sb = per.tile([P, NWE], fp16, tag="off2")
            iota_rep = per.tile([P, KB * P], fp16, tag="iota")
            P_sb = per.tile([1, NWN * P], f32, tag="Psb")

            nc.sync.dma_start(xe[:], d_xfm[:])
            nc.sync.dma_start(off1_sb[:], d_off1[:])
            nc.sync.dma_start(gidx_sb[:], d_gidx[:])
            nc.sync.dma_start(off2_sb[:], d_off2[:])
            nc.gpsimd.iota(iota_rep[:], pattern=[[0, KB], [1, P]], base=0,
                           channel_multiplier=0,
                           allow_small_or_imprecise_dtypes=True)

            def silu_ps(out_bf, ps_in, bias):
                """out = silu(ps_in + bias); bias is AP or 0.0"""
                if not sim_safe:
                    nc.scalar.activation(out_bf, ps_in, SILU, bias=bias, scale=1.0)
                else:
                    zt = sbp.tile(list(out_bf.shape), bf16, tag="ss_z")
                    st = sbp.tile(list(out_bf.shape), bf16, tag="ss_s")
                    nc.scalar.activation(zt[:], ps_in, AF.Identity, bias=bias, scale=1.0)
                    nc.scalar.activation(st[:], ps_in, AF.Sigmoid, bias=bias, scale=1.0)
                    nc.vector.tensor_tensor(out=out_bf, in0=zt[:], in1=st[:],
                                            op=mybir.AluOpType.mult)

            # ---------------- per-block weight loads
            def load_block_weights(b):
                w = {}
                for nm, shp in [("Wkj", [H, H]), ("Wji", [H, H]), ("Wlin", [H, H]),
                                ("Wdown", [H, IE]), ("Wup", [IE, H]),
                                ("W12r", [NR, H]), ("W12s", [NSR, IE])]:
                    t = cst.tile(shp, bf16, tag=nm)
                    nc.sync.dma_start(t[:], wd[nm][b])
                    w[nm] = t
                tw = cst.tile([H, 6 * H], bf16, tag="Wres")
                tb = cst.tile([H, 6], f32, tag="bres")
                for r in range(3):
                    for s in range(2):
                        k = r * 2 + s
                        nc.sync.dma_start(tw[:, k * H:(k + 1) * H], wd["Wres"][b, r, s])
                        nc.sync.dma_start(tb[:, k:k + 1], wd["bres"][b, r, s])
                w["Wres"] = lambda r, s: tw[:, (r * 2 + s) * H:(r * 2 + s + 1) * H]
                w["bres"] = lambda r, s: tb[:, (r * 2 + s):(r * 2 + s) + 1]
                for nm in ["bkj", "bji", "blin"]:
                    t = cst.tile([H, 1], f32, tag=nm)
                    nc.sync.dma_start(t[:], wd[nm][b])
                    w[nm] = t
                return w

            def load_out_weights(ob):
                w = {}
                t = cst.tile([NR, H], bf16, tag="Worbf")
                nc.sync.dma_start(t[:], wd["Worbf"][ob]); w["Worbf"] = t
                t = cst.tile([H, OE], bf16, tag="Woup")
                nc.sync.dma_start(t[:], wd["Woup"][ob])
                w["Woup"] = lambda hf: t[:, hf * 128:(hf + 1) * 128]
                tt = cst.tile([H, 3 * 2 * OE], bf16, tag="Wouts")
                for l in range(3):
                    for hin in range(2):
                        k = l * 2 + hin
                        nc.sync.dma_start(tt[:, k * OE:(k + 1) * OE],
                                          wd["Wouts"][ob, l, hin])
                # lhsT block [128(in half), 128(out half)]
                w["Wouts"] = lambda l, hin, hf: tt[
                    :, (l * 2 + hin) * OE + hf * 128:(l * 2 + hin) * OE + hf * 128 + 128]
                t2 = cst.tile([H, 2], bf16, tag="Wo")
                for hf in range(2):
                    nc.sync.dma_start(t2[:, hf:hf + 1], wd["Wo"][ob, hf])
                w["Wo"] = lambda hf: t2[:, hf:hf + 1]
                t3 = cst.tile([H, 2], f32, tag="boup")
                for hf in range(2):
                    nc.sync.dma_start(t3[:, hf:hf + 1],
                                      wd["boup"][ob, hf * 128:(hf + 1) * 128])
                w["boup"] = lambda hf: t3[:, hf:hf + 1]
                t4 = cst.tile([H, 6], f32, tag="bouts")
                for l in range(3):
                    for hf in range(2):
                        nc.sync.dma_start(t4[:, l * 2 + hf:l * 2 + hf + 1],
                                          wd["bouts"][ob, l, hf * 128:(hf + 1) * 128])
                w["bouts"] = lambda l, hf: t4[:, l * 2 + hf:l * 2 + hf + 1]
                return w

            def mm512(ps, lhsT, rhs_ap, width):
                """matmul over <width> moving cols in 512 slices."""
                for s in range(0, width, 512):
                    e = min(s + 512, width)
                    nc.tensor.matmul(ps[:, s:e], lhsT, rhs_ap[:, s:e],
                                     start=True, stop=True)

            # ---------------- output block
            def out_block(ob, first):
                w = load_out_weights(ob)
                t_bf = hch.tile([H, NWN * P], bf16, tag="tbf")
                for nw in range(NWN):
                    t_ps = pbig.tile([H, P], f32, tag="pb")
                    for t0 in range(0, C2, 4):
                        tb = min(4, C2 - t0)
                        rbf_sl = sbp.tile([NR, 4 * P], bf16, tag="rbfsl")
                        nc.sync.dma_start(
                            rbf_sl[:, :tb * P],
                            d_rbfT[:, (nw * C2 + t0) * P:(nw * C2 + t0 + tb) * P])
                        xer = sbp.tile([P, tb * H], bf16, tag="xer")
                        ps_o = pbig.tile([P, tb * H], f32, tag="pb")
                        for k in range(tb):
                            te = nw * C2 + t0 + k
                            if first:
                                nc.sync.dma_start(
                                    xer[:, k * H:(k + 1) * H],
                                    d_xrm[te * P:(te + 1) * P, :])
                            else:
                                nc.sync.dma_start_transpose(
                                    xer[:, k * H:(k + 1) * H],
                                    xe[:, te * P:(te + 1) * P])
                            nc.tensor.matmul(
                                ps_o[:, k * H:(k + 1) * H],
                                rbf_sl[:, k * P:(k + 1) * P],
                                w["Worbf"][:], start=True, stop=True)
                        o_sb = sbp.tile([P, tb * H], bf16, tag="osb")
                        nc.vector.tensor_tensor(out=o_sb[:], in0=xer[:],
                                                in1=ps_o[:],
                                                op=mybir.AluOpType.mult)
                        B2 = sbp.tile([P, tb * P], bf16, tag="B2")
                        nc.vector.tensor_tensor(
                            out=B2[:].rearrange("p (j f) -> p j f", f=P),
                            in0=off2_sb[:, nw * C2 + t0: nw * C2 + t0 + tb]
                                .to_broadcast([P, tb, P]),
                            in1=iota_rep[:, :tb * P].rearrange(
                                "p (j f) -> p j f", f=P),
                            op=mybir.AluOpType.is_equal)
                        for k in range(tb):
                            nc.tensor.matmul(
                                t_ps[:], o_sb[:, k * H:(k + 1) * H],
                                B2[:, k * P:(k + 1) * P],
                                start=(t0 + k == 0), stop=(t0 + k == C2 - 1))
                    nc.vector.tensor_copy(t_bf[:, nw * P:(nw + 1) * P], t_ps[:])

                # node MLP
                for s0 in range(0, NWN * P, 512):
                    sw = min(512, NWN * P - s0)
                    z = []
                    for hf in range(2):
                        psM = pbig.tile([P, 512], f32, tag="pb")
                        nc.tensor.matmul(psM[:, :sw], w["Woup"](hf),
                                         t_bf[:, s0:s0 + sw], start=True, stop=True)
                        y = hch.tile([P, 512], bf16, tag=f"y{hf}")
                        nc.scalar.activation(y[:, :sw], psM[:, :sw], AF.Identity,
                                             bias=w["boup"](hf), scale=1.0)
                        z.append(y)
                    for l in range(3):
                        z2 = []
                        for hf in range(2):
                            psM = pbig.tile([P, 512], f32, tag="pb")
                            for hin in range(2):
                                nc.tensor.matmul(
                                    psM[:, :sw], w["Wouts"](l, hin, hf),
                                    z[hin][:, :sw],
                                    start=(hin == 0), stop=(hin == 1))
                            y = hch.tile([P, 512], bf16, tag=f"z{l % 2}{hf}")
                            silu_ps(y[:, :sw], psM[:, :sw], w["bouts"](l, hf))
                            z2.append(y)
                        z = z2
                    psP = pbig.tile([1, 512], f32, tag="pb")
                    for hf in range(2):
                        nc.tensor.matmul(psP[:, :sw], w["Wo"](hf), z[hf][:, :sw],
                                         start=(hf == 0), stop=(hf == 1))
                    if first:
                        nc.vector.tensor_copy(P_sb[:, s0:s0 + sw], psP[:, :sw])
                    else:
                        nc.vector.tensor_add(P_sb[:, s0:s0 + sw],
                                             P_sb[:, s0:s0 + sw], psP[:, :sw])

            # ---------------- phase 1: x_kj_down for all edges
            def phase1(b, w):
                for g0 in range(0, NWE, GW1):
                    gw = min(GW1, NWE - g0)
                    cols = slice(g0 * P, (g0 + gw) * P)
                    wid = gw * P
                    psA = pbig.tile([H, GW1 * P], f32, tag="pb")
                    mm512(psA[:, :wid], w["Wkj"][:], xe[:, cols], wid)
                    t1 = sbp.tile([H, GW1 * P], bf16, tag="t1")
                    silu_ps(t1[:, :wid], psA[:, :wid], w["bkj"][:])
                    rbf_sl = sbp.tile([NR, GW1 * P], bf16, tag="rbfsl1")
                    nc.sync.dma_start(rbf_sl[:, :wid], d_rbfT[:, cols])
                    psB = pbig.tile([H, GW1 * P], f32, tag="pb")
                    mm512(psB[:, :wid], w["W12r"][:], rbf_sl[:, :wid], wid)
                    xkj = sbp.tile([H, GW1 * P], bf16, tag="xkj")
                    nc.vector.tensor_tensor(out=xkj[:, :wid], in0=t1[:, :wid],
                                            in1=psB[:, :wid],
                                            op=mybir.AluOpType.mult)
                    psC = pbig.tile([P, GW1 * IE], f32, tag="pb")
                    for t in range(gw):
                        nc.tensor.matmul(psC[:, t * IE:(t + 1) * IE],
                                         xkj[:, t * P:(t + 1) * P],
                                         w["Wdown"][:], start=True, stop=True)
                    xkjd = sbp.tile([P, GW1 * IE], bf16, tag="xkjd")
                    silu_ps(xkjd[:, :gw * IE], psC[:, :gw * IE], 0.0)
                    nc.sync.dma_start(
                        ag_in[g0 * P:(g0 + gw) * P, :].rearrange(
                            "(t p) f -> p t f", p=P),
                        xkjd[:, :gw * IE].rearrange("p (t f) -> p t f", f=IE))

            # ---------------- phase 2+3: triplet aggregation + h chain
            def phase23(b, w):
                for g0 in range(0, NWE, GW3):
                    gw = min(GW3, NWE - g0)
                    cols = slice(g0 * P, (g0 + gw) * P)
                    wid = gw * P
                    agg_g = sbp.tile([IE, GW3 * P], bf16, tag="aggg")
                    ch_lo, ch_hi = g0 * C1, (g0 + gw) * C1
                    agg_tiles = {}
                    for c0 in range(ch_lo, ch_hi, KB):
                        kb = min(KB, ch_hi - c0)
                        xg = sbp.tile([P, KB * IE], bf16, tag="xg")
                        # HW indirect DMA only honors one offset per
                        # partition per call -> one call per chunk.
                        for j in range(kb):
                            nc.gpsimd.indirect_dma_start(
                                out=xg[:, j * IE:(j + 1) * IE],
                                out_offset=None,
                                in_=ag_out,
                                in_offset=bass.IndirectOffsetOnAxis(
                                    ap=gidx_sb[:, c0 + j:c0 + j + 1], axis=0))
                        sbf_sl = sbp.tile([NSR, KB * P], bf16, tag="sbfsl")
                        nc.sync.dma_start(sbf_sl[:, :kb * P],
                                          d_sbfT[:, c0 * P:(c0 + kb) * P])
                        ps_s = pbig.tile([P, KB * IE], f32, tag="pb")
                        for j in range(kb):
                            nc.tensor.matmul(ps_s[:, j * IE:(j + 1) * IE],
                                             sbf_sl[:, j * P:(j + 1) * P],
                                             w["W12s"][:], start=True, stop=True)
                        m_sb = sbp.tile([P, KB * IE], bf16, tag="msb")
                        nc.vector.tensor_tensor(out=m_sb[:, :kb * IE],
                                                in0=xg[:, :kb * IE],
                                                in1=ps_s[:, :kb * IE],
                                                op=mybir.AluOpType.mult)
                        Bsb = sbp.tile([P, KB * P], bf16, tag="Bsb")
                        nc.vector.tensor_tensor(
                            out=Bsb[:, :kb * P].rearrange("p (j f) -> p j f", f=P),
                            in0=off1_sb[:, c0:c0 + kb].to_broadcast([P, kb, P]),
                            in1=iota_rep[:, :kb * P].rearrange("p (j f) -> p j f", f=P),
                            op=mybir.AluOpType.is_equal)
                        for j in range(kb):
                            ch = c0 + j
                            w_abs = ch // C1
                            c = ch % C1
                            w_rel = w_abs - g0
                            sub = w_rel // 4
                            if sub not in agg_tiles:
                                agg_tiles[sub] = pagg.tile([IE, 512], f32, tag="agg", name="aggps")
                            nc.tensor.matmul(
                                agg_tiles[sub][:, (w_rel % 4) * P:(w_rel % 4 + 1) * P],
                                m_sb[:, j * IE:(j + 1) * IE],
                                Bsb[:, j * P:(j + 1) * P],
                                start=(c == 0), stop=(c == C1 - 1))
                    for sub, tl in agg_tiles.items():
                        sw = min(512, wid - sub * 512)
                        nc.vector.tensor_copy(
                            agg_g[:, sub * 512: sub * 512 + sw], tl[:, :sw])

                    # ---- h chain on this group
                    psD = pbig.tile([H, GW3 * P], f32, tag="pb")
                    mm512(psD[:, :wid], w["Wji"][:], xe[:, cols], wid)
                    xji = hch.tile([H, GW3 * P], bf16, tag="xji")
                    silu_ps(xji[:, :wid], psD[:, :wid], w["bji"][:])

                    psD = pbig.tile([H, GW3 * P], f32, tag="pb")
                    mm512(psD[:, :wid], w["Wup"][:], agg_g[:, :wid], wid)
                    hbuf = hch.tile([H, GW3 * P], bf16, tag="hbuf")
                    silu_ps(hbuf[:, :wid], psD[:, :wid], 0.0)
                    nc.vector.tensor_add(hbuf[:, :wid], hbuf[:, :wid], xji[:, :wid])

                    def res_layer(hin, r):
                        psX = pbig.tile([H, GW3 * P], f32, tag="pb")
                        mm512(psX[:, :wid], w["Wres"](r, 0), hin[:, :wid], wid)
                        tt = hch.tile([H, GW3 * P], bf16, tag="res_t")
                        silu_ps(tt[:, :wid], psX[:, :wid], w["bres"](r, 0))
                        psX = pbig.tile([H, GW3 * P], f32, tag="pb")
                        mm512(psX[:, :wid], w["Wres"](r, 1), tt[:, :wid], wid)
                        t2 = hch.tile([H, GW3 * P], bf16, tag="res_t2")
                        silu_ps(t2[:, :wid], psX[:, :wid], w["bres"](r, 1))
                        nc.vector.tensor_add(hin[:, :wid], hin[:, :wid], t2[:, :wid])

                    res_layer(hbuf, 0)
                    psD = pbig.tile([H, GW3 * P], f32, tag="pb")
                    mm512(psD[:, :wid], w["Wlin"][:], hbuf[:, :wid], wid)
                    t3 = hch.tile([H, GW3 * P], bf16, tag="t3")
                    silu_ps(t3[:, :wid], psD[:, :wid], w["blin"][:])
                    h2 = hch.tile([H, GW3 * P], bf16, tag="h2")
                    nc.vector.tensor_add(h2[:, :wid], t3[:, :wid], xe[:, cols])
                    res_layer(h2, 1)
                    res_layer(h2, 2)
                    nc.vector.tensor_copy(xe[:, cols], h2[:, :wid])

            # ---------------- main program
            out_block(0, first=True)
            for b in range(NB):
                w = load_block_weights(b)
                phase1(b, w)
                nc.gpsimd.collective_compute(
                    "AllGather", mybir.AluOpType.bypass,
                    replica_groups=[list(range(ncores))],
                    ins=[ag_in], outs=[ag_out])
                phase23(b, w)
                out_block(b + 1, first=False)

            nc.sync.dma_start(d_P[:], P_sb[:])

    nc.compile()
    return nc


# ---------------------------------------------------------------- entry
def run_kernel(inputs, ncores=8, sim=False, trace=False, prebuilt=None):
    if sim:
        os.environ.setdefault("JAX_PLATFORMS", "cpu")
    else:
        os.environ["JAX_PLATFORMS"] = "axon"
    import concourse.bass_utils as bass_utils
    from concourse import bacc

    x = np.asarray(inputs["x"], np.float32)
    rbf = np.asarray(inputs["rbf"], np.float32)
    sbf = np.asarray(inputs["sbf"], np.float32)
    dims, data = host_prep(x, rbf, sbf, inputs["idx_kj"], inputs["idx_ji"],
                           inputs["idx_i"], inputs["num_nodes"], ncores)
    ws = prep_weights(inputs)

    if prebuilt is None:
        nc = bacc.Bacc("TRN2", target_bir_lowering=False, debug=False,
                       enable_asserts=False, num_devices=ncores)
        build_kernel(nc, dims, sim_safe=sim)
    else:
        nc = prebuilt

    in_maps = []
    for c in range(ncores):
        m = {k: np.ascontiguousarray(v[c]) for k, v in data.items()}
        m.update(ws)
        in_maps.append(m)

    if sim:
        from concourse.bass_interp import MultiCoreSim
        simu = MultiCoreSim(nc, num_cores=ncores)
        for c in range(ncores):
            for k, v in in_maps[c].items():
                simu.cores[c].tensor(k)[:] = v
        simu.simulate()
        outs = [np.array(simu.cores[c].tensor("P_out")) for c in range(ncores)]
        res = None
    else:
        res = bass_utils.run_bass_kernel_spmd(
            nc, in_maps, core_ids=list(range(ncores)), trace=trace,
            stitch_traces=False)
        outs = [res.results[c]["P_out"] for c in range(ncores)]

    N = int(inputs["num_nodes"])
    Pfull = np.zeros((N, OC), np.float32)
    n_lo = dims["n_lo"]; nodes_c = dims["nodes_c"]
    for c in range(ncores):
        Pfull[n_lo[c]: n_lo[c] + nodes_c[c], 0] = outs[c].reshape(-1)[: nodes_c[c]]
    return Pfull, res, dims


def kernel(**inputs) -> np.ndarray:
    out, _, _ = run_kernel(inputs, ncores=8, sim=False, trace=False)
    return out


# revision 10
# speedup vs baseline: 1.0038x; 1.0038x over previous
"""DimeNet++ Trainium2 kernel (8 NeuronCores, graph-parallel).

Self-contained: takes full inputs, shards internally, runs one SPMD Bass
program on 8 cores, gathers the full output.

Layout strategy (per core c of 8):
 - Edges sorted by idx_i (target node), nodes split into 8 contiguous
   ranges with ~equal edge counts. Each core owns its node range and the
   edges pointing into it. Edge stream padded so every 128-node window
   owns exactly C2 edge tiles of 128 (E_pad = NWN*C2*128).
 - Triplets assigned to the core owning their destination edge (idx_ji),
   sorted by destination edge window (128 edges), padded so every window
   owns exactly C1 chunks of 128 triplet rows (T_pad = NWE*C1*128).
 - Activations kept feature-major ([H=128 partitions, edges]) in SBUF
   across all 4 interaction blocks; x_kj_down is all-gathered (collective)
   across cores each block; triplet gather via indirect DMA; both
   segment-sums (idx_ji and idx_i) are done as one-hot matmuls built from
   iota/is_equal compares, accumulating in PSUM.
"""

import math
import os

import numpy as np

H, IE, BE, NS, NR, OE, OC = 128, 64, 8, 7, 6, 256, 1
NB = 4
NSR = NS * NR  # 42
P = 128


# ---------------------------------------------------------------- host prep
def _ceil_div(a, b):
    return -(-a // b)


def host_prep(x, rbf, sbf, idx_kj, idx_ji, idx_i, num_nodes, ncores):
    import ml_dtypes

    bf = ml_dtypes.bfloat16
    E = x.shape[0]
    T = sbf.shape[0]
    N = int(num_nodes)
    idx_i = np.asarray(idx_i, np.int64)
    idx_kj = np.asarray(idx_kj, np.int64)
    idx_ji = np.asarray(idx_ji, np.int64)

    # ---- core node boundaries, balancing edge counts
    counts_n = np.bincount(idx_i, minlength=N)
    cumn = np.concatenate([[0], np.cumsum(counts_n)])  # cumn[n] = #edges node < n
    bounds = [0]
    for c in range(1, ncores):
        bounds.append(int(np.searchsorted(cumn, c * E / ncores)))
    bounds.append(N)
    n_lo = np.array(bounds[:-1])
    n_hi = np.array(bounds[1:])
    nodes_c = n_hi - n_lo
    NWN = int(max(_ceil_div(int(nc_), P) for nc_ in nodes_c))

    # ---- C2: edge tiles per 128-node window
    C2 = 1
    win_edge_cnt = np.zeros((ncores, NWN), np.int64)
    for c in range(ncores):
        for w in range(NWN):
            a = min(int(n_lo[c]) + P * w, int(n_hi[c]))
            b = min(a + P, int(n_hi[c]))
            cnt = int(cumn[b] - cumn[a])
            win_edge_cnt[c, w] = cnt
            C2 = max(C2, _ceil_div(cnt, P))
    E_pad = NWN * C2 * P
    NWE = NWN * C2

    # ---- edge placement
    eperm = np.argsort(idx_i, kind="stable")  # edges ordered by node
    g2l = np.full(E, -1, np.int64)
    edge_list = np.full((ncores, E_pad), -1, np.int64)
    for c in range(ncores):
        for w in range(NWN):
            a = min(int(n_lo[c]) + P * w, int(n_hi[c]))
            b = min(a + P, int(n_hi[c]))
            lo, hi = int(cumn[a]), int(cumn[b])
            cnt = hi - lo
            base = w * C2 * P
            edge_list[c, base : base + cnt] = eperm[lo:hi]
            g2l[eperm[lo:hi]] = c * E_pad + base + np.arange(cnt)

    x_fm = np.zeros((ncores, H, E_pad), bf)
    x_rm = np.zeros((ncores, E_pad, H), bf)
    rbfT = np.zeros((ncores, NR, E_pad), bf)
    off2 = np.zeros((ncores, P, NWE), np.float16)
    for c in range(ncores):
        el = edge_list[c]
        v = el >= 0
        xr = np.zeros((E_pad, H), np.float32)
        xr[v] = x[el[v]]
        x_rm[c] = xr.astype(bf)
        x_fm[c] = xr.T.astype(bf)
        rr = np.zeros((E_pad, NR), np.float32)
        rr[v] = rbf[el[v]]
        rbfT[c] = rr.T.astype(bf)
        o2 = np.zeros(E_pad, np.float16)
        nw = np.arange(E_pad) // (C2 * P)  # node window of each slot
        o2[v] = (idx_i[el[v]] - (n_lo[c] + P * nw[v])).astype(np.float16)
        off2[c] = o2.reshape(NWE, P).T

    # ---- triplets
    dest_g = g2l[idx_ji]
    src_g = g2l[idx_kj]
    assert dest_g.min() >= 0 and src_g.min() >= 0
    dest_c = dest_g // E_pad
    dest_l = dest_g % E_pad

    # C1: chunks per 128-edge window
    C1 = 1
    per_core = []
    for c in range(ncores):
        m = dest_c == c
        dl = dest_l[m]
        sg = src_g[m]
        rows = np.nonzero(m)[0]
        win = dl >> 7
        order = np.lexsort((sg, win))
        dl, sg, rows, win = dl[order], sg[order], rows[order], win[order]
        wcnt = np.bincount(win, minlength=NWE)
        C1 = max(C1, int(_ceil_div(int(wcnt.max()), P)))
        per_core.append((dl, sg, rows, win, wcnt))

    T_pad = NWE * C1 * P
    NCHUNK = NWE * C1
    sbfT = np.zeros((ncores, NSR, T_pad), bf)
    gidx = np.zeros((ncores, P, NCHUNK), np.int32)
    off1 = np.zeros((ncores, P, NCHUNK), np.float16)
    for c in range(ncores):
        dl, sg, rows, win, wcnt = per_core[c]
        wstart = np.concatenate([[0], np.cumsum(wcnt)])[:-1]
        pos = win * (C1 * P) + (np.arange(len(dl)) - wstart[win])
        sp = np.zeros((T_pad, NSR), np.float32)
        sp[pos] = sbf[rows]
        sbfT[c] = sp.T.astype(bf)
        gi = np.zeros(T_pad, np.int32)
        gi[pos] = sg.astype(np.int32)
        gidx[c] = gi.reshape(NCHUNK, P).T
        o1 = np.zeros(T_pad, np.float16)
        o1[pos] = (dl & 127).astype(np.float16)
        off1[c] = o1.reshape(NCHUNK, P).T

    dims = dict(
        E_pad=E_pad, NWE=NWE, NWN=NWN, C1=C1, C2=C2, T_pad=T_pad,
        NCHUNK=NCHUNK, ncores=ncores, n_lo=n_lo.tolist(), nodes_c=nodes_c.tolist(),
    )
    data = dict(x_fm=x_fm, x_rm=x_rm, rbfT=rbfT, off2=off2, sbfT=sbfT,
                gidx=gidx, off1=off1)
    return dims, data


def prep_weights(ws):
    """ws: dict of the weight arrays from setup_inputs (numpy float32)."""
    import ml_dtypes

    bf = ml_dtypes.bfloat16
    f32 = np.float32
    out = {}
    out["Wkj"] = ws["Wkj"].astype(bf)                    # [NB,128,128]
    out["Wji"] = ws["Wji"].astype(bf)
    out["Wlin"] = ws["Wlin"].astype(bf)
    out["Wres"] = ws["Wres"].astype(bf)                  # [NB,3,2,128,128]
    out["Wdown"] = ws["Wdown"].astype(bf)                # [NB,128,64]
    out["Wup"] = ws["Wup"].astype(bf)                    # [NB,64,128]
    out["W12r"] = np.einsum("bij,bjk->bik", ws["Wrbf1"], ws["Wrbf2"]).astype(bf)   # [NB,6,128]
    out["W12s"] = np.einsum("bij,bjk->bik", ws["Wsbf1"], ws["Wsbf2"]).astype(bf)   # [NB,42,64]
    out["Worbf"] = ws["Worbf"].astype(bf)                # [NB+1,6,128]
    out["Woup"] = ws["Woup"].astype(bf)                  # [NB+1,128,256]
    # Wouts [NB+1,3,256,256] -> [NB+1,3,2(in half),128,256]
    out["Wouts"] = ws["Wouts"].reshape(NB + 1, 3, 2, 128, OE).astype(bf)
    out["Wo"] = ws["Wo"].reshape(NB + 1, 2, 128, OC).astype(bf)   # [NB+1,2,128,1]
    out["bkj"] = ws["bkj"].reshape(NB, H, 1).astype(f32)
    out["bji"] = ws["bji"].reshape(NB, H, 1).astype(f32)
    out["blin"] = ws["blin"].reshape(NB, H, 1).astype(f32)
    out["bres"] = ws["bres"].reshape(NB, 3, 2, H, 1).astype(f32)
    out["boup"] = ws["boup"].reshape(NB + 1, OE, 1).astype(f32)   # [NB+1,256,1]
    out["bouts"] = ws["bouts"].reshape(NB + 1, 3, OE, 1).astype(f32)
    return out


# ---------------------------------------------------------------- bass build
def build_kernel(nc, dims, sim_safe=False):
    import concourse.bass as bass
    import concourse.tile as tile
    from concourse import mybir

    f32 = mybir.dt.float32
    bf16 = mybir.dt.bfloat16
    fp16 = mybir.dt.float16
    i32 = mybir.dt.int32
    AF = mybir.ActivationFunctionType
    SILU = AF.Sigmoid if sim_safe else AF.Silu

    E_pad = dims["E_pad"]; NWE = dims["NWE"]; NWN = dims["NWN"]
    C1 = dims["C1"]; C2 = dims["C2"]; T_pad = dims["T_pad"]
    NCHUNK = dims["NCHUNK"]; ncores = dims["ncores"]

    # ---------------- dram I/O
    d_xfm = nc.dram_tensor("x_fm", [H, E_pad], bf16, kind="ExternalInput").ap()
    d_xrm = nc.dram_tensor("x_rm", [E_pad, H], bf16, kind="ExternalInput").ap()
    d_rbfT = nc.dram_tensor("rbfT", [NR, E_pad], bf16, kind="ExternalInput").ap()
    d_sbfT = nc.dram_tensor("sbfT", [NSR, T_pad], bf16, kind="ExternalInput").ap()
    d_gidx = nc.dram_tensor("gidx", [P, NCHUNK], i32, kind="ExternalInput").ap()
    d_off1 = nc.dram_tensor("off1", [P, NCHUNK], fp16, kind="ExternalInput").ap()
    d_off2 = nc.dram_tensor("off2", [P, NWE], fp16, kind="ExternalInput").ap()

    wd = {}
    for nm, shp, dt in [
        ("Wkj", [NB, H, H], bf16), ("Wji", [NB, H, H], bf16),
        ("Wlin", [NB, H, H], bf16), ("Wres", [NB, 3, 2, H, H], bf16),
        ("Wdown", [NB, H, IE], bf16), ("Wup", [NB, IE, H], bf16),
        ("W12r", [NB, NR, H], bf16), ("W12s", [NB, NSR, IE], bf16),
        ("Worbf", [NB + 1, NR, H], bf16), ("Woup", [NB + 1, H, OE], bf16),
        ("Wouts", [NB + 1, 3, 2, 128, OE], bf16), ("Wo", [NB + 1, 2, 128, OC], bf16),
        ("bkj", [NB, H, 1], f32), ("bji", [NB, H, 1], f32),
        ("blin", [NB, H, 1], f32), ("bres", [NB, 3, 2, H, 1], f32),
        ("boup", [NB + 1, OE, 1], f32), ("bouts", [NB + 1, 3, OE, 1], f32),
    ]:
        wd[nm] = nc.dram_tensor(nm, shp, dt, kind="ExternalInput").ap()

    d_P = nc.dram_tensor("P_out", [1, NWN * P], f32, kind="ExternalOutput").ap()

    ag_in = nc.dram_tensor("ag_in", [E_pad, IE], bf16, kind="Internal").ap()
    ag_out = nc.dram_tensor(
        "ag_out", [ncores * E_pad, IE], bf16, kind="Internal",
        addr_space="Shared" if ncores > 4 else "Local",
    ).ap()

    KB = 8  # chunk batch in triplet phase
    GW1 = 8   # tiles per phase-1 group (1024 edge cols)
    GW3 = 8   # windows per phase-2/3 group

    with tile.TileContext(nc) as tc:
        import contextlib
        ctx = contextlib.ExitStack()
        with ctx:
            per = ctx.enter_context(tc.tile_pool(name="per", bufs=1))
            cst = ctx.enter_context(tc.tile_pool(name="cst", bufs=1))
            sbp = ctx.enter_context(tc.tile_pool(name="sbp", bufs=2))
            hch = ctx.enter_context(tc.tile_pool(name="hch", bufs=2))
            pbig = ctx.enter_context(tc.tile_pool(name="pbig", bufs=3, space="PSUM"))
            pagg = ctx.enter_context(tc.tile_pool(name="pagg", bufs=2, space="PSUM"))

            # ---------------- persistent tiles
            xe = per.tile([H, E_pad], bf16, tag="xe")
            off1_sb = per.tile([P, NCHUNK], fp16, tag="off1")
            gidx_sb = per.tile([P, NCHUNK], i32, tag="gidx")
            off2_sb = per.tile([P, NWE], fp16, tag="off2")
            iota_rep = per.tile([P, KB * P], fp16, tag="iota")
            P_sb = per.tile([1, NWN * P], f32, tag="Psb")

            nc.sync.dma_start(xe[:], d_xfm[:])
            nc.sync.dma_start(off1_sb[:], d_off1[:])
            nc.sync.dma_start(gidx_sb[:], d_gidx[:])
            nc.sync.dma_start(off2_sb[:], d_off2[:])
            nc.gpsimd.iota(iota_rep[:], pattern=[[0, KB], [1, P]], base=0,
                           channel_multiplier=0,
                           allow_small_or_imprecise_dtypes=True)

            def silu_ps(out_bf, ps_in, bias):
                """out = silu(ps_in + bias); bias is AP or 0.0"""
                if not sim_safe:
                    nc.scalar.activation(out_bf, ps_in, SILU, bias=bias, scale=1.0)
                else:
                    zt = sbp.tile(list(out_bf.shape), bf16, tag="ss_z")
                    st = sbp.tile(list(out_bf.shape), bf16, tag="ss_s")
                    nc.scalar.activation(zt[:], ps_in, AF.Identity, bias=bias, scale=1.0)
                    nc.scalar.activation(st[:], ps_in, AF.Sigmoid, bias=bias, scale=1.0)
                    nc.vector.tensor_tensor(out=out_bf, in0=zt[:], in1=st[:],
                                            op=mybir.AluOpType.mult)

            # ---------------- per-block weight loads
            def load_block_weights(b):
                w = {}
                for nm, shp in [("Wkj", [H, H]), ("Wji", [H, H]), ("Wlin", [H, H]),
                                ("Wdown", [H, IE]), ("Wup", [IE, H]),
                                ("W12r", [NR, H]), ("W12s", [NSR, IE])]:
                    t = cst.tile(shp, bf16, tag=nm)
                    nc.sync.dma_start(t[:], wd[nm][b])
                    w[nm] = t
                tw = cst.tile([H, 6 * H], bf16, tag="Wres")
                tb = cst.tile([H, 6], f32, tag="bres")
                for r in range(3):
                    for s in range(2):
                        k = r * 2 + s
                        nc.sync.dma_start(tw[:, k * H:(k + 1) * H], wd["Wres"][b, r, s])
                        nc.sync.dma_start(tb[:, k:k + 1], wd["bres"][b, r, s])
                w["Wres"] = lambda r, s: tw[:, (r * 2 + s) * H:(r * 2 + s + 1) * H]
                w["bres"] = lambda r, s: tb[:, (r * 2 + s):(r * 2 + s) + 1]
                for nm in ["bkj", "bji", "blin"]:
                    t = cst.tile([H, 1], f32, tag=nm)
                    nc.sync.dma_start(t[:], wd[nm][b])
                    w[nm] = t
                return w

            def load_out_weights(ob):
                w = {}
                t = cst.tile([NR, H], bf16, tag="Worbf")
                nc.sync.dma_start(t[:], wd["Worbf"][ob]); w["Worbf"] = t
                t = cst.tile([H, OE], bf16, tag="Woup")
                nc.sync.dma_start(t[:], wd["Woup"][ob])
                w["Woup"] = lambda hf: t[:, hf * 128:(hf + 1) * 128]
                tt = cst.tile([H, 3 * 2 * OE], bf16, tag="Wouts")
                for l in range(3):
                    for hin in range(2):
                        k = l * 2 + hin
                        nc.sync.dma_start(tt[:, k * OE:(k + 1) * OE],
                                          wd["Wouts"][ob, l, hin])
                # lhsT block [128(in half), 128(out half)]
                w["Wouts"] = lambda l, hin, hf: tt[
                    :, (l * 2 + hin) * OE + hf * 128:(l * 2 + hin) * OE + hf * 128 + 128]
                t2 = cst.tile([H, 2], bf16, tag="Wo")
                for hf in range(2):
                    nc.sync.dma_start(t2[:, hf:hf + 1], wd["Wo"][ob, hf])
                w["Wo"] = lambda hf: t2[:, hf:hf + 1]
                t3 = cst.tile([H, 2], f32, tag="boup")
                for hf in range(2):
                    nc.sync.dma_start(t3[:, hf:hf + 1],
                                      wd["boup"][ob, hf * 128:(hf + 1) * 128])
                w["boup"] = lambda hf: t3[:, hf:hf + 1]
                t4 = cst.tile([H, 6], f32, tag="bouts")
                for l in range(3):
                    for hf in range(2):
                        nc.sync.dma_start(t4[:, l * 2 + hf:l * 2 + hf + 1],
                                          wd["bouts"][ob, l, hf * 128:(hf + 1) * 128])
                w["bouts"] = lambda l, hf: t4[:, l * 2 + hf:l * 2 + hf + 1]
                return w

            def mm512(ps, lhsT, rhs_ap, width):
                """matmul over <width> moving cols in 512 slices."""
                for s in range(0, width, 512):
                    e = min(s + 512, width)
                    nc.tensor.matmul(ps[:, s:e], lhsT, rhs_ap[:, s:e],
                                     start=True, stop=True)

            # ---------------- output block
            def out_block(ob, first):
                w = load_out_weights(ob)
                t_bf = hch.tile([H, NWN * P], bf16, tag="tbf")
                for nw in range(NWN):
                    t_ps = pbig.tile([H, P], f32, tag="pb")
                    for t0 in range(0, C2, 4):
                        tb = min(4, C2 - t0)
                        rbf_sl = sbp.tile([NR, 4 * P], bf16, tag="rbfsl")
                        nc.sync.dma_start(
                            rbf_sl[:, :tb * P],
                            d_rbfT[:, (nw * C2 + t0) * P:(nw * C2 + t0 + tb) * P])
                        xer = sbp.tile([P, tb * H], bf16, tag="xer")
                        ps_o = pbig.tile([P, tb * H], f32, tag="pb")
                        for k in range(tb):
                            te = nw * C2 + t0 + k
                            if first:
                                nc.sync.dma_start(
                                    xer[:, k * H:(k + 1) * H],
                                    d_xrm[te * P:(te + 1) * P, :])
                            else:
                                nc.sync.dma_start_transpose(
                                    xer[:, k * H:(k + 1) * H],
                                    xe[:, te * P:(te + 1) * P])
                            nc.tensor.matmul(
                                ps_o[:, k * H:(k + 1) * H],
                                rbf_sl[:, k * P:(k + 1) * P],
                                w["Worbf"][:], start=True, stop=True)
                        o_sb = sbp.tile([P, tb * H], bf16, tag="osb")
                        nc.vector.tensor_tensor(out=o_sb[:], in0=xer[:],
                                                in1=ps_o[:],
                                                op=mybir.AluOpType.mult)
                        B2 = sbp.tile([P, tb * P], bf16, tag="B2")
                        nc.vector.tensor_tensor(
                            out=B2[:].rearrange("p (j f) -> p j f", f=P),
                            in0=off2_sb[:, nw * C2 + t0: nw * C2 + t0 + tb]
                                .to_broadcast([P, tb, P]),
                            in1=iota_rep[:, :tb * P].rearrange(
                                "p (j f) -> p j f", f=P),
                            op=mybir.AluOpType.is_equal)
                        for k in range(tb):
                            nc.tensor.matmul(
                                t_ps[:], o_sb[:, k * H:(k + 1) * H],
                                B2[:, k * P:(k + 1) * P],
                                start=(t0 + k == 0), stop=(t0 + k == C2 - 1))
                    nc.vector.tensor_copy(t_bf[:, nw * P:(nw + 1) * P], t_ps[:])

                # node MLP
                for s0 in range(0, NWN * P, 512):
                    sw = min(512, NWN * P - s0)
                    z = []
                    for hf in range(2):
                        psM = pbig.tile([P, 512], f32, tag="pb")
                        nc.tensor.matmul(psM[:, :sw], w["Woup"](hf),
                                         t_bf[:, s0:s0 + sw], start=True, stop=True)
                        y = hch.tile([P, 512], bf16, tag=f"y{hf}")
                        nc.scalar.activation(y[:, :sw], psM[:, :sw], AF.Identity,
                                             bias=w["boup"](hf), scale=1.0)
                        z.append(y)
                    for l in range(3):
                        z2 = []
                        for hf in range(2):
                            psM = pbig.tile([P, 512], f32, tag="pb")
                            for hin in range(2):
                                nc.tensor.matmul(
                                    psM[:, :sw], w["Wouts"](l, hin, hf),
                                    z[hin][:, :sw],
                                    start=(hin == 0), stop=(hin == 1))
                            y = hch.tile([P, 512], bf16, tag=f"z{l % 2}{hf}")
                            silu_ps(y[:, :sw], psM[:, :sw], w["bouts"](l, hf))
                            z2.append(y)
                        z = z2
                    psP = pbig.tile([1, 512], f32, tag="pb")
                    for hf in range(2):
                        nc.tensor.matmul(psP[:, :sw], w["Wo"](hf), z[hf][:, :sw],
                                         start=(hf == 0), stop=(hf == 1))
                    if first:
                        nc.vector.tensor_copy(P_sb[:, s0:s0 + sw], psP[:, :sw])
                    else:
                        nc.vector.tensor_add(P_sb[:, s0:s0 + sw],
                                             P_sb[:, s0:s0 + sw], psP[:, :sw])

            # ---------------- phase 1: x_kj_down for all edges
            def phase1(b, w):
                for g0 in range(0, NWE, GW1):
                    gw = min(GW1, NWE - g0)
                    cols = slice(g0 * P, (g0 + gw) * P)
                    wid = gw * P
                    psA = pbig.tile([H, GW1 * P], f32, tag="pb")
                    mm512(psA[:, :wid], w["Wkj"][:], xe[:, cols], wid)
                    t1 = sbp.tile([H, GW1 * P], bf16, tag="t1")
                    silu_ps(t1[:, :wid], psA[:, :wid], w["bkj"][:])
                    rbf_sl = sbp.tile([NR, GW1 * P], bf16, tag="rbfsl1")
                    nc.sync.dma_start(rbf_sl[:, :wid], d_rbfT[:, cols])
                    psB = pbig.tile([H, GW1 * P], f32, tag="pb")
                    mm512(psB[:, :wid], w["W12r"][:], rbf_sl[:, :wid], wid)
                    xkj = sbp.tile([H, GW1 * P], bf16, tag="xkj")
                    nc.vector.tensor_tensor(out=xkj[:, :wid], in0=t1[:, :wid],
                                            in1=psB[:, :wid],
                                            op=mybir.AluOpType.mult)
                    psC = pbig.tile([P, GW1 * IE], f32, tag="pb")
                    for t in range(gw):
                        nc.tensor.matmul(psC[:, t * IE:(t + 1) * IE],
                                         xkj[:, t * P:(t + 1) * P],
                                         w["Wdown"][:], start=True, stop=True)
                    xkjd = sbp.tile([P, GW1 * IE], bf16, tag="xkjd")
                    silu_ps(xkjd[:, :gw * IE], psC[:, :gw * IE], 0.0)
                    nc.sync.dma_start(
                        ag_in[g0 * P:(g0 + gw) * P, :].rearrange(
                            "(t p) f -> p t f", p=P),
                        xkjd[:, :gw * IE].rearrange("p (t f) -> p t f", f=IE))

            # ---------------- phase 2+3: triplet aggregation + h chain
            def phase23(b, w):
                for g0 in range(0, NWE, GW3):
                    gw = min(GW3, NWE - g0)
                    cols = slice(g0 * P, (g0 + gw) * P)
                    wid = gw * P
                    agg_g = sbp.tile([IE, GW3 * P], bf16, tag="aggg")
                    ch_lo, ch_hi = g0 * C1, (g0 + gw) * C1
                    gcn = ch_hi - ch_lo
                    agg_tiles = {}
                    # prefetch the whole group's gathers so the Pool engine
                    # can run ahead of the compute phases (one [P,1]
                    # indirect call per chunk -- HW limit).
                    xg = sbp.tile([P, GW3 * C1 * IE], bf16, tag="xg")
                    for j in range(gcn):
                        nc.gpsimd.indirect_dma_start(
                            out=xg[:, j * IE:(j + 1) * IE],
                            out_offset=None,
                            in_=ag_out,
                            in_offset=bass.IndirectOffsetOnAxis(
                                ap=gidx_sb[:, ch_lo + j:ch_lo + j + 1], axis=0))
                    for c0 in range(ch_lo, ch_hi, KB):
                        kb = min(KB, ch_hi - c0)
                        xo = (c0 - ch_lo) * IE
                        sbf_sl = sbp.tile([NSR, KB * P], bf16, tag="sbfsl")
                        nc.sync.dma_start(sbf_sl[:, :kb * P],
                                          d_sbfT[:, c0 * P:(c0 + kb) * P])
                        ps_s = pbig.tile([P, KB * IE], f32, tag="pb")
                        for j in range(kb):
                            nc.tensor.matmul(ps_s[:, j * IE:(j + 1) * IE],
                                             sbf_sl[:, j * P:(j + 1) * P],
                                             w["W12s"][:], start=True, stop=True)
                        m_sb = sbp.tile([P, KB * IE], bf16, tag="msb")
                        nc.vector.tensor_tensor(out=m_sb[:, :kb * IE],
                                                in0=xg[:, xo:xo + kb * IE],
                                                in1=ps_s[:, :kb * IE],
                                                op=mybir.AluOpType.mult)
                        Bsb = sbp.tile([P, KB * P], bf16, tag="Bsb")
                        nc.vector.tensor_tensor(
                            out=Bsb[:, :kb * P].rearrange("p (j f) -> p j f", f=P),
                            in0=off1_sb[:, c0:c0 + kb].to_broadcast([P, kb, P]),
                            in1=iota_rep[:, :kb * P].rearrange("p (j f) -> p j f", f=P),
                            op=mybir.AluOpType.is_equal)
                        for j in range(kb):
                            ch = c0 + j
                            w_abs = ch // C1
                            c = ch % C1
                            w_rel = w_abs - g0
                            sub = w_rel // 4
                            if sub not in agg_tiles:
                                agg_tiles[sub] = pagg.tile([IE, 512], f32, tag="agg", name="aggps")
                            nc.tensor.matmul(
                                agg_tiles[sub][:, (w_rel % 4) * P:(w_rel % 4 + 1) * P],
                                m_sb[:, j * IE:(j + 1) * IE],
                                Bsb[:, j * P:(j + 1) * P],
                                start=(c == 0), stop=(c == C1 - 1))
                    for sub, tl in agg_tiles.items():
                        sw = min(512, wid - sub * 512)
                        nc.vector.tensor_copy(
                            agg_g[:, sub * 512: sub * 512 + sw], tl[:, :sw])

                    # ---- h chain on this group
                    psD = pbig.tile([H, GW3 * P], f32, tag="pb")
                    mm512(psD[:, :wid], w["Wji"][:], xe[:, cols], wid)
                    xji = hch.tile([H, GW3 * P], bf16, tag="xji")
                    silu_ps(xji[:, :wid], psD[:, :wid], w["bji"][:])

                    psD = pbig.tile([H, GW3 * P], f32, tag="pb")
                    mm512(psD[:, :wid], w["Wup"][:], agg_g[:, :wid], wid)
                    hbuf = hch.tile([H, GW3 * P], bf16, tag="hbuf")
                    silu_ps(hbuf[:, :wid], psD[:, :wid], 0.0)
                    nc.vector.tensor_add(hbuf[:, :wid], hbuf[:, :wid], xji[:, :wid])

                    def res_layer(hin, r):
                        psX = pbig.tile([H, GW3 * P], f32, tag="pb")
                        mm512(psX[:, :wid], w["Wres"](r, 0), hin[:, :wid], wid)
                        tt = hch.tile([H, GW3 * P], bf16, tag="res_t")
                        silu_ps(tt[:, :wid], psX[:, :wid], w["bres"](r, 0))
                        psX = pbig.tile([H, GW3 * P], f32, tag="pb")
                        mm512(psX[:, :wid], w["Wres"](r, 1), tt[:, :wid], wid)
                        t2 = hch.tile([H, GW3 * P], bf16, tag="res_t2")
                        silu_ps(t2[:, :wid], psX[:, :wid], w["bres"](r, 1))
                        nc.vector.tensor_add(hin[:, :wid], hin[:, :wid], t2[:, :wid])

                    res_layer(hbuf, 0)
                    psD = pbig.tile([H, GW3 * P], f32, tag="pb")
                    mm512(psD[:, :wid], w["Wlin"][:], hbuf[:, :wid], wid)
                    t3 = hch.tile([H, GW3 * P], bf16, tag="t3")
                    silu_ps(t3[:, :wid], psD[:, :wid], w["blin"][:])
                    h2 = hch.tile([H, GW3 * P], bf16, tag="h2")
                    nc.vector.tensor_add(h2[:, :wid], t3[:, :wid], xe[:, cols])
                    res_layer(h2, 1)
                    res_layer(h2, 2)
                    nc.vector.tensor_copy(xe[:, cols], h2[:, :wid])

            # ---------------- main program
            out_block(0, first=True)
            for b in range(NB):
                w = load_block_weights(b)
                phase1(b, w)
                nc.gpsimd.collective_compute(
                    "AllGather", mybir.AluOpType.bypass,
                    replica_groups=[list(range(ncores))],
                    ins=[ag_in], outs=[ag_out])
                phase23(b, w)
                out_block(b + 1, first=False)

            nc.sync.dma_start(d_P[:], P_sb[:])

    nc.compile()
    return nc


# ---------------------------------------------------------------- entry
def run_kernel(inputs, ncores=8, sim=False, trace=False, prebuilt=None):
    if sim:
        os.environ.setdefault("JAX_PLATFORMS", "cpu")
    else:
        os.environ["JAX_PLATFORMS"] = "axon"
    import concourse.bass_utils as bass_utils
    from concourse import bacc

    x = np.asarray(inputs["x"], np.float32)
    rbf = np.asarray(inputs["rbf"], np.float32)
    sbf = np.asarray(inputs["sbf"], np.float32)
    dims, data = host_prep(x, rbf, sbf, inputs["idx_kj"], inputs["idx_ji"],
                           inputs["idx_i"], inputs["num_nodes"], ncores)
    ws = prep_weights(inputs)

    if prebuilt is None:
        nc = bacc.Bacc("TRN2", target_bir_lowering=False, debug=False,
                       enable_asserts=False, num_devices=ncores)
        build_kernel(nc, dims, sim_safe=sim)
    else:
        nc = prebuilt

    in_maps = []
    for c in range(ncores):
        m = {k: np.ascontiguousarray(v[c]) for k, v in data.items()}
        m.update(ws)
        in_maps.append(m)

    if sim:
        from concourse.bass_interp import MultiCoreSim
        simu = MultiCoreSim(nc, num_cores=ncores)
        for c in range(ncores):
            for k, v in in_maps[c].items():
                simu.cores[c].tensor(k)[:] = v
        simu.simulate()
        outs = [np.array(simu.cores[c].tensor("P_out")) for c in range(ncores)]
        res = None
    else:
        res = bass_utils.run_bass_kernel_spmd(
            nc, in_maps, core_ids=list(range(ncores)), trace=trace,
            stitch_traces=False)
        outs = [res.results[c]["P_out"] for c in range(ncores)]

    N = int(inputs["num_nodes"])
    Pfull = np.zeros((N, OC), np.float32)
    n_lo = dims["n_lo"]; nodes_c = dims["nodes_c"]
    for c in range(ncores):
        Pfull[n_lo[c]: n_lo[c] + nodes_c[c], 0] = outs[c].reshape(-1)[: nodes_c[c]]
    return Pfull, res, dims


def kernel(**inputs) -> np.ndarray:
    out, _, _ = run_kernel(inputs, ncores=8, sim=False, trace=False)
    return out


# revision 13
# speedup vs baseline: 1.0343x; 1.0304x over previous
"""DimeNet++ Trainium2 kernel (8 NeuronCores, graph-parallel).

Self-contained: takes full inputs, shards internally, runs one SPMD Bass
program on 8 cores, gathers the full output.

Layout strategy (per core c of 8):
 - Edges sorted by idx_i (target node), nodes split into 8 contiguous
   ranges with ~equal edge counts. Each core owns its node range and the
   edges pointing into it. Edge stream padded so every 128-node window
   owns exactly C2 edge tiles of 128 (E_pad = NWN*C2*128).
 - Triplets assigned to the core owning their destination edge (idx_ji),
   sorted by destination edge window (128 edges), padded so every window
   owns exactly C1 chunks of 128 triplet rows (T_pad = NWE*C1*128).
 - Activations kept feature-major ([H=128 partitions, edges]) in SBUF
   across all 4 interaction blocks; x_kj_down is all-gathered (collective)
   across cores each block; triplet gather via indirect DMA; both
   segment-sums (idx_ji and idx_i) are done as one-hot matmuls built from
   iota/is_equal compares, accumulating in PSUM.
"""

import math
import os

import numpy as np

H, IE, BE, NS, NR, OE, OC = 128, 64, 8, 7, 6, 256, 1
NB = 4
NSR = NS * NR  # 42
P = 128


# ---------------------------------------------------------------- host prep
def _ceil_div(a, b):
    return -(-a // b)


def host_prep(x, rbf, sbf, idx_kj, idx_ji, idx_i, num_nodes, ncores):
    import ml_dtypes

    bf = ml_dtypes.bfloat16
    E = x.shape[0]
    T = sbf.shape[0]
    N = int(num_nodes)
    idx_i = np.asarray(idx_i, np.int64)
    idx_kj = np.asarray(idx_kj, np.int64)
    idx_ji = np.asarray(idx_ji, np.int64)

    # ---- core node boundaries, balancing edge counts
    counts_n = np.bincount(idx_i, minlength=N)
    cumn = np.concatenate([[0], np.cumsum(counts_n)])  # cumn[n] = #edges node < n
    bounds = [0]
    for c in range(1, ncores):
        bounds.append(int(np.searchsorted(cumn, c * E / ncores)))
    bounds.append(N)
    n_lo = np.array(bounds[:-1])
    n_hi = np.array(bounds[1:])
    nodes_c = n_hi - n_lo
    NWN = int(max(_ceil_div(int(nc_), P) for nc_ in nodes_c))

    # ---- C2: edge tiles per 128-node window
    C2 = 1
    win_edge_cnt = np.zeros((ncores, NWN), np.int64)
    for c in range(ncores):
        for w in range(NWN):
            a = min(int(n_lo[c]) + P * w, int(n_hi[c]))
            b = min(a + P, int(n_hi[c]))
            cnt = int(cumn[b] - cumn[a])
            win_edge_cnt[c, w] = cnt
            C2 = max(C2, _ceil_div(cnt, P))
    if (NWN * C2) % 2:
        NWN += 1  # keep NWE even so 256-wide scatter windows always apply
    E_pad = NWN * C2 * P
    NWE = NWN * C2

    # ---- edge placement
    eperm = np.argsort(idx_i, kind="stable")  # edges ordered by node
    g2l = np.full(E, -1, np.int64)
    edge_list = np.full((ncores, E_pad), -1, np.int64)
    for c in range(ncores):
        for w in range(NWN):
            a = min(int(n_lo[c]) + P * w, int(n_hi[c]))
            b = min(a + P, int(n_hi[c]))
            lo, hi = int(cumn[a]), int(cumn[b])
            cnt = hi - lo
            base = w * C2 * P
            edge_list[c, base : base + cnt] = eperm[lo:hi]
            g2l[eperm[lo:hi]] = c * E_pad + base + np.arange(cnt)

    x_fm = np.zeros((ncores, H, E_pad), bf)
    x_rm = np.zeros((ncores, E_pad, H), bf)
    rbfT = np.zeros((ncores, NR, E_pad), bf)
    off2 = np.zeros((ncores, P, NWE), np.float16)
    for c in range(ncores):
        el = edge_list[c]
        v = el >= 0
        xr = np.zeros((E_pad, H), np.float32)
        xr[v] = x[el[v]]
        x_rm[c] = xr.astype(bf)
        x_fm[c] = xr.T.astype(bf)
        rr = np.zeros((E_pad, NR), np.float32)
        rr[v] = rbf[el[v]]
        rbfT[c] = rr.T.astype(bf)
        o2 = np.zeros(E_pad, np.float16)
        nw = np.arange(E_pad) // (C2 * P)  # node window of each slot
        o2[v] = (idx_i[el[v]] - (n_lo[c] + P * nw[v])).astype(np.float16)
        off2[c] = o2.reshape(NWE, P).T

    # ---- triplets
    dest_g = g2l[idx_ji]
    src_g = g2l[idx_kj]
    assert dest_g.min() >= 0 and src_g.min() >= 0
    dest_c = dest_g // E_pad
    dest_l = dest_g % E_pad

    # scatter windows: 256 edges wide when NWE is even, else 128
    WSH = 8 if NWE % 2 == 0 else 7
    WW = 1 << WSH                      # window width in edges
    NWS = NWE * P // WW                # number of scatter windows
    C1 = 1
    per_core = []
    for c in range(ncores):
        m = dest_c == c
        dl = dest_l[m]
        sg = src_g[m]
        rows = np.nonzero(m)[0]
        win = dl >> WSH
        order = np.lexsort((sg, win))
        dl, sg, rows, win = dl[order], sg[order], rows[order], win[order]
        wcnt = np.bincount(win, minlength=NWS)
        C1 = max(C1, int(_ceil_div(int(wcnt.max()), P)))
        per_core.append((dl, sg, rows, win, wcnt))

    T_pad = NWS * C1 * P
    NCHUNK = NWS * C1
    sbfT = np.zeros((ncores, NSR, T_pad), bf)
    gidx = np.zeros((ncores, P, NCHUNK), np.int32)
    off1 = np.zeros((ncores, P, NCHUNK), np.float16)
    for c in range(ncores):
        dl, sg, rows, win, wcnt = per_core[c]
        wstart = np.concatenate([[0], np.cumsum(wcnt)])[:-1]
        pos = win * (C1 * P) + (np.arange(len(dl)) - wstart[win])
        sp = np.zeros((T_pad, NSR), np.float32)
        sp[pos] = sbf[rows]
        sbfT[c] = sp.T.astype(bf)
        gi = np.zeros(T_pad, np.int32)
        gi[pos] = sg.astype(np.int32)
        gidx[c] = gi.reshape(NCHUNK, P).T
        o1 = np.zeros(T_pad, np.float16)
        o1[pos] = (dl & (WW - 1)).astype(np.float16)
        off1[c] = o1.reshape(NCHUNK, P).T

    dims = dict(
        E_pad=E_pad, NWE=NWE, NWN=NWN, C1=C1, C2=C2, T_pad=T_pad,
        NCHUNK=NCHUNK, WW=WW, NWS=NWS, ncores=ncores, n_lo=n_lo.tolist(),
        nodes_c=nodes_c.tolist(),
    )
    data = dict(x_fm=x_fm, x_rm=x_rm, rbfT=rbfT, off2=off2, sbfT=sbfT,
                gidx=gidx, off1=off1)
    return dims, data


def prep_weights(ws):
    """ws: dict of the weight arrays from setup_inputs (numpy float32)."""
    import ml_dtypes

    bf = ml_dtypes.bfloat16
    f32 = np.float32
    out = {}
    out["Wkj"] = ws["Wkj"].astype(bf)                    # [NB,128,128]
    out["Wji"] = ws["Wji"].astype(bf)
    out["Wlin"] = ws["Wlin"].astype(bf)
    out["Wres"] = ws["Wres"].astype(bf)                  # [NB,3,2,128,128]
    out["Wdown"] = ws["Wdown"].astype(bf)                # [NB,128,64]
    out["Wup"] = ws["Wup"].astype(bf)                    # [NB,64,128]
    out["W12r"] = np.einsum("bij,bjk->bik", ws["Wrbf1"], ws["Wrbf2"]).astype(bf)   # [NB,6,128]
    out["W12s"] = np.einsum("bij,bjk->bik", ws["Wsbf1"], ws["Wsbf2"]).astype(bf)   # [NB,42,64]
    out["Worbf"] = ws["Worbf"].astype(bf)                # [NB+1,6,128]
    out["Woup"] = ws["Woup"].astype(bf)                  # [NB+1,128,256]
    # Wouts [NB+1,3,256,256] -> [NB+1,3,2(in half),128,256]
    out["Wouts"] = ws["Wouts"].reshape(NB + 1, 3, 2, 128, OE).astype(bf)
    out["Wo"] = ws["Wo"].reshape(NB + 1, 2, 128, OC).astype(bf)   # [NB+1,2,128,1]
    out["bkj"] = ws["bkj"].reshape(NB, H, 1).astype(f32)
    out["bji"] = ws["bji"].reshape(NB, H, 1).astype(f32)
    out["blin"] = ws["blin"].reshape(NB, H, 1).astype(f32)
    out["bres"] = ws["bres"].reshape(NB, 3, 2, H, 1).astype(f32)
    out["boup"] = ws["boup"].reshape(NB + 1, OE, 1).astype(f32)   # [NB+1,256,1]
    out["bouts"] = ws["bouts"].reshape(NB + 1, 3, OE, 1).astype(f32)
    return out


# ---------------------------------------------------------------- bass build
def build_kernel(nc, dims, sim_safe=False):
    import concourse.bass as bass
    import concourse.tile as tile
    from concourse import mybir

    f32 = mybir.dt.float32
    bf16 = mybir.dt.bfloat16
    fp16 = mybir.dt.float16
    i32 = mybir.dt.int32
    AF = mybir.ActivationFunctionType
    SILU = AF.Sigmoid if sim_safe else AF.Silu

    E_pad = dims["E_pad"]; NWE = dims["NWE"]; NWN = dims["NWN"]
    C1 = dims["C1"]; C2 = dims["C2"]; T_pad = dims["T_pad"]
    NCHUNK = dims["NCHUNK"]; ncores = dims["ncores"]
    WW = dims["WW"]; NWS = dims["NWS"]; WPT = WW // P  # tiles per scatter window

    # ---------------- dram I/O
    d_xfm = nc.dram_tensor("x_fm", [H, E_pad], bf16, kind="ExternalInput").ap()
    d_xrm = nc.dram_tensor("x_rm", [E_pad, H], bf16, kind="ExternalInput").ap()
    d_rbfT = nc.dram_tensor("rbfT", [NR, E_pad], bf16, kind="ExternalInput").ap()
    d_sbfT = nc.dram_tensor("sbfT", [NSR, T_pad], bf16, kind="ExternalInput").ap()
    d_gidx = nc.dram_tensor("gidx", [P, NCHUNK], i32, kind="ExternalInput").ap()
    d_off1 = nc.dram_tensor("off1", [P, NCHUNK], fp16, kind="ExternalInput").ap()
    d_off2 = nc.dram_tensor("off2", [P, NWE], fp16, kind="ExternalInput").ap()

    wd = {}
    for nm, shp, dt in [
        ("Wkj", [NB, H, H], bf16), ("Wji", [NB, H, H], bf16),
        ("Wlin", [NB, H, H], bf16), ("Wres", [NB, 3, 2, H, H], bf16),
        ("Wdown", [NB, H, IE], bf16), ("Wup", [NB, IE, H], bf16),
        ("W12r", [NB, NR, H], bf16), ("W12s", [NB, NSR, IE], bf16),
        ("Worbf", [NB + 1, NR, H], bf16), ("Woup", [NB + 1, H, OE], bf16),
        ("Wouts", [NB + 1, 3, 2, 128, OE], bf16), ("Wo", [NB + 1, 2, 128, OC], bf16),
        ("bkj", [NB, H, 1], f32), ("bji", [NB, H, 1], f32),
        ("blin", [NB, H, 1], f32), ("bres", [NB, 3, 2, H, 1], f32),
        ("boup", [NB + 1, OE, 1], f32), ("bouts", [NB + 1, 3, OE, 1], f32),
    ]:
        wd[nm] = nc.dram_tensor(nm, shp, dt, kind="ExternalInput").ap()

    d_P = nc.dram_tensor("P_out", [1, NWN * P], f32, kind="ExternalOutput").ap()

    ag_in = nc.dram_tensor("ag_in", [E_pad, IE], bf16, kind="Internal").ap()
    ag_out = nc.dram_tensor(
        "ag_out", [ncores * E_pad, IE], bf16, kind="Internal",
        addr_space="Shared" if ncores > 4 else "Local",
    ).ap()

    KB = 4  # chunk batch in triplet phase
    GW1 = 8   # tiles per phase-1 group (1024 edge cols)
    GW3 = 8   # windows per phase-2/3 group

    with tile.TileContext(nc) as tc:
        import contextlib
        ctx = contextlib.ExitStack()
        with ctx:
            per = ctx.enter_context(tc.tile_pool(name="per", bufs=1))
            cst = ctx.enter_context(tc.tile_pool(name="cst", bufs=1))
            sbp = ctx.enter_context(tc.tile_pool(name="sbp", bufs=2))
            hch = ctx.enter_context(tc.tile_pool(name="hch", bufs=2))
            pbig = ctx.enter_context(tc.tile_pool(name="pbig", bufs=3, space="PSUM"))
            pagg = ctx.enter_context(tc.tile_pool(name="pagg", bufs=2, space="PSUM"))

            # ---------------- persistent tiles
            xe = per.tile([H, E_pad], bf16, tag="xe")
            off1_sb = per.tile([P, NCHUNK], fp16, tag="off1")
            gidx_sb = per.tile([P, NCHUNK], i32, tag="gidx")
            off2_sb = per.tile([P, NWE], fp16, tag="off2")
            iota_rep = per.tile([P, KB * WW], fp16, tag="iota")
            P_sb = per.tile([1, NWN * P], f32, tag="Psb")

            nc.sync.dma_start(xe[:], d_xfm[:])
            nc.sync.dma_start(off1_sb[:], d_off1[:])
            nc.sync.dma_start(gidx_sb[:], d_gidx[:])
            nc.sync.dma_start(off2_sb[:], d_off2[:])
            nc.gpsimd.iota(iota_rep[:], pattern=[[0, KB], [1, WW]], base=0,
                           channel_multiplier=0,
                           allow_small_or_imprecise_dtypes=True)

            def silu_ps(out_bf, ps_in, bias):
                """out = silu(ps_in + bias); bias is AP or 0.0"""
                if not sim_safe:
                    nc.scalar.activation(out_bf, ps_in, SILU, bias=bias, scale=1.0)
                else:
                    zt = sbp.tile(list(out_bf.shape), bf16, tag="ss_z")
                    st = sbp.tile(list(out_bf.shape), bf16, tag="ss_s")
                    nc.scalar.activation(zt[:], ps_in, AF.Identity, bias=bias, scale=1.0)
                    nc.scalar.activation(st[:], ps_in, AF.Sigmoid, bias=bias, scale=1.0)
                    nc.vector.tensor_tensor(out=out_bf, in0=zt[:], in1=st[:],
                                            op=mybir.AluOpType.mult)

            # ---------------- per-block weight loads
            def load_block_weights(b):
                w = {}
                for nm, shp in [("Wkj", [H, H]), ("Wji", [H, H]), ("Wlin", [H, H]),
                                ("Wdown", [H, IE]), ("Wup", [IE, H]),
                                ("W12r", [NR, H]), ("W12s", [NSR, IE])]:
                    t = cst.tile(shp, bf16, tag=nm)
                    nc.sync.dma_start(t[:], wd[nm][b])
                    w[nm] = t
                tw = cst.tile([H, 6 * H], bf16, tag="Wres")
                tb = cst.tile([H, 6], f32, tag="bres")
                for r in range(3):
                    for s in range(2):
                        k = r * 2 + s
                        nc.sync.dma_start(tw[:, k * H:(k + 1) * H], wd["Wres"][b, r, s])
                        nc.sync.dma_start(tb[:, k:k + 1], wd["bres"][b, r, s])
                w["Wres"] = lambda r, s: tw[:, (r * 2 + s) * H:(r * 2 + s + 1) * H]
                w["bres"] = lambda r, s: tb[:, (r * 2 + s):(r * 2 + s) + 1]
                for nm in ["bkj", "bji", "blin"]:
                    t = cst.tile([H, 1], f32, tag=nm)
                    nc.sync.dma_start(t[:], wd[nm][b])
                    w[nm] = t
                return w

            def load_out_weights(ob):
                w = {}
                t = cst.tile([NR, H], bf16, tag="Worbf")
                nc.sync.dma_start(t[:], wd["Worbf"][ob]); w["Worbf"] = t
                t = cst.tile([H, OE], bf16, tag="Woup")
                nc.sync.dma_start(t[:], wd["Woup"][ob])
                w["Woup"] = lambda hf: t[:, hf * 128:(hf + 1) * 128]
                tt = cst.tile([H, 3 * 2 * OE], bf16, tag="Wouts")
                for l in range(3):
                    for hin in range(2):
                        k = l * 2 + hin
                        nc.sync.dma_start(tt[:, k * OE:(k + 1) * OE],
                                          wd["Wouts"][ob, l, hin])
                # lhsT block [128(in half), 128(out half)]
                w["Wouts"] = lambda l, hin, hf: tt[
                    :, (l * 2 + hin) * OE + hf * 128:(l * 2 + hin) * OE + hf * 128 + 128]
                t2 = cst.tile([H, 2], bf16, tag="Wo")
                for hf in range(2):
                    nc.sync.dma_start(t2[:, hf:hf + 1], wd["Wo"][ob, hf])
                w["Wo"] = lambda hf: t2[:, hf:hf + 1]
                t3 = cst.tile([H, 2], f32, tag="boup")
                for hf in range(2):
                    nc.sync.dma_start(t3[:, hf:hf + 1],
                                      wd["boup"][ob, hf * 128:(hf + 1) * 128])
                w["boup"] = lambda hf: t3[:, hf:hf + 1]
                t4 = cst.tile([H, 6], f32, tag="bouts")
                for l in range(3):
                    for hf in range(2):
                        nc.sync.dma_start(t4[:, l * 2 + hf:l * 2 + hf + 1],
                                          wd["bouts"][ob, l, hf * 128:(hf + 1) * 128])
                w["bouts"] = lambda l, hf: t4[:, l * 2 + hf:l * 2 + hf + 1]
                return w

            def mm512(ps, lhsT, rhs_ap, width):
                """matmul over <width> moving cols in 512 slices."""
                for s in range(0, width, 512):
                    e = min(s + 512, width)
                    nc.tensor.matmul(ps[:, s:e], lhsT, rhs_ap[:, s:e],
                                     start=True, stop=True)

            # ---------------- output block
            def out_block(ob, first):
                w = load_out_weights(ob)
                t_bf = hch.tile([H, NWN * P], bf16, tag="tbf")
                for nw in range(NWN):
                    t_ps = pbig.tile([H, P], f32, tag="pb")
                    for t0 in range(0, C2, 4):
                        tb = min(4, C2 - t0)
                        rbf_sl = sbp.tile([NR, 4 * P], bf16, tag="rbfsl")
                        nc.sync.dma_start(
                            rbf_sl[:, :tb * P],
                            d_rbfT[:, (nw * C2 + t0) * P:(nw * C2 + t0 + tb) * P])
                        xer = sbp.tile([P, tb * H], bf16, tag="xer")
                        ps_o = pbig.tile([P, tb * H], f32, tag="pb")
                        for k in range(tb):
                            te = nw * C2 + t0 + k
                            if first:
                                nc.sync.dma_start(
                                    xer[:, k * H:(k + 1) * H],
                                    d_xrm[te * P:(te + 1) * P, :])
                            else:
                                nc.sync.dma_start_transpose(
                                    xer[:, k * H:(k + 1) * H],
                                    xe[:, te * P:(te + 1) * P])
                            nc.tensor.matmul(
                                ps_o[:, k * H:(k + 1) * H],
                                rbf_sl[:, k * P:(k + 1) * P],
                                w["Worbf"][:], start=True, stop=True)
                        o_sb = sbp.tile([P, tb * H], bf16, tag="osb")
                        nc.vector.tensor_tensor(out=o_sb[:], in0=xer[:],
                                                in1=ps_o[:],
                                                op=mybir.AluOpType.mult)
                        B2 = sbp.tile([P, tb * P], bf16, tag="B2")
                        for k2 in range(tb):
                            nc.vector.tensor_tensor(
                                out=B2[:, k2 * P:(k2 + 1) * P],
                                in0=off2_sb[:, nw * C2 + t0 + k2:
                                            nw * C2 + t0 + k2 + 1]
                                    .to_broadcast([P, P]),
                                in1=iota_rep[:, :P],
                                op=mybir.AluOpType.is_equal)
                        for k in range(tb):
                            nc.tensor.matmul(
                                t_ps[:], o_sb[:, k * H:(k + 1) * H],
                                B2[:, k * P:(k + 1) * P],
                                start=(t0 + k == 0), stop=(t0 + k == C2 - 1))
                    nc.vector.tensor_copy(t_bf[:, nw * P:(nw + 1) * P], t_ps[:])

                # node MLP
                for s0 in range(0, NWN * P, 512):
                    sw = min(512, NWN * P - s0)
                    z = []
                    for hf in range(2):
                        psM = pbig.tile([P, 512], f32, tag="pb")
                        nc.tensor.matmul(psM[:, :sw], w["Woup"](hf),
                                         t_bf[:, s0:s0 + sw], start=True, stop=True)
                        y = hch.tile([P, 512], bf16, tag=f"y{hf}")
                        nc.scalar.activation(y[:, :sw], psM[:, :sw], AF.Identity,
                                             bias=w["boup"](hf), scale=1.0)
                        z.append(y)
                    for l in range(3):
                        z2 = []
                        for hf in range(2):
                            psM = pbig.tile([P, 512], f32, tag="pb")
                            for hin in range(2):
                                nc.tensor.matmul(
                                    psM[:, :sw], w["Wouts"](l, hin, hf),
                                    z[hin][:, :sw],
                                    start=(hin == 0), stop=(hin == 1))
                            y = hch.tile([P, 512], bf16, tag=f"z{l % 2}{hf}")
                            silu_ps(y[:, :sw], psM[:, :sw], w["bouts"](l, hf))
                            z2.append(y)
                        z = z2
                    psP = pbig.tile([1, 512], f32, tag="pb")
                    for hf in range(2):
                        nc.tensor.matmul(psP[:, :sw], w["Wo"](hf), z[hf][:, :sw],
                                         start=(hf == 0), stop=(hf == 1))
                    if first:
                        nc.vector.tensor_copy(P_sb[:, s0:s0 + sw], psP[:, :sw])
                    else:
                        nc.vector.tensor_add(P_sb[:, s0:s0 + sw],
                                             P_sb[:, s0:s0 + sw], psP[:, :sw])

            # ---------------- phase 1: x_kj_down for all edges
            def phase1(b, w):
                for g0 in range(0, NWE, GW1):
                    gw = min(GW1, NWE - g0)
                    cols = slice(g0 * P, (g0 + gw) * P)
                    wid = gw * P
                    psA = pbig.tile([H, GW1 * P], f32, tag="pb")
                    mm512(psA[:, :wid], w["Wkj"][:], xe[:, cols], wid)
                    t1 = sbp.tile([H, GW1 * P], bf16, tag="t1")
                    silu_ps(t1[:, :wid], psA[:, :wid], w["bkj"][:])
                    rbf_sl = sbp.tile([NR, GW1 * P], bf16, tag="rbfsl1")
                    nc.sync.dma_start(rbf_sl[:, :wid], d_rbfT[:, cols])
                    psB = pbig.tile([H, GW1 * P], f32, tag="pb")
                    mm512(psB[:, :wid], w["W12r"][:], rbf_sl[:, :wid], wid)
                    xkj = sbp.tile([H, GW1 * P], bf16, tag="xkj")
                    nc.vector.tensor_tensor(out=xkj[:, :wid], in0=t1[:, :wid],
                                            in1=psB[:, :wid],
                                            op=mybir.AluOpType.mult)
                    psC = pbig.tile([P, GW1 * IE], f32, tag="pb")
                    for t in range(gw):
                        nc.tensor.matmul(psC[:, t * IE:(t + 1) * IE],
                                         xkj[:, t * P:(t + 1) * P],
                                         w["Wdown"][:], start=True, stop=True)
                    xkjd = sbp.tile([P, GW1 * IE], bf16, tag="xkjd")
                    silu_ps(xkjd[:, :gw * IE], psC[:, :gw * IE], 0.0)
                    nc.sync.dma_start(
                        ag_in[g0 * P:(g0 + gw) * P, :].rearrange(
                            "(t p) f -> p t f", p=P),
                        xkjd[:, :gw * IE].rearrange("p (t f) -> p t f", f=IE))

            # ---------------- phase 2+3: triplet aggregation + h chain
            def phase23(b, w):
                for g0 in range(0, NWE, GW3):
                    gw = min(GW3, NWE - g0)
                    cols = slice(g0 * P, (g0 + gw) * P)
                    wid = gw * P
                    agg_g = sbp.tile([IE, GW3 * P], bf16, tag="aggg")
                    assert (g0 * P) % WW == 0 and (gw * P) % WW == 0
                    ws_lo = g0 * P // WW
                    ch_lo = ws_lo * C1
                    ch_hi = (ws_lo + gw * P // WW) * C1
                    gcn = ch_hi - ch_lo
                    agg_tiles = {}
                    # prefetch the whole group's gathers so the Pool engine
                    # can run ahead of the compute phases (one [P,1]
                    # indirect call per chunk -- HW limit).
                    xg = sbp.tile([P, (GW3 * P // WW) * C1 * IE], bf16, tag="xg")
                    for j in range(gcn):
                        nc.gpsimd.indirect_dma_start(
                            out=xg[:, j * IE:(j + 1) * IE],
                            out_offset=None,
                            in_=ag_out,
                            in_offset=bass.IndirectOffsetOnAxis(
                                ap=gidx_sb[:, ch_lo + j:ch_lo + j + 1], axis=0))
                    for c0 in range(ch_lo, ch_hi, KB):
                        kb = min(KB, ch_hi - c0)
                        xo = (c0 - ch_lo) * IE
                        sbf_sl = sbp.tile([NSR, KB * P], bf16, tag="sbfsl")
                        nc.sync.dma_start(sbf_sl[:, :kb * P],
                                          d_sbfT[:, c0 * P:(c0 + kb) * P])
                        ps_s = pbig.tile([P, KB * IE], f32, tag="pb")
                        for j in range(kb):
                            nc.tensor.matmul(ps_s[:, j * IE:(j + 1) * IE],
                                             sbf_sl[:, j * P:(j + 1) * P],
                                             w["W12s"][:], start=True, stop=True)
                        m_sb = sbp.tile([P, KB * IE], bf16, tag="msb")
                        nc.vector.tensor_tensor(out=m_sb[:, :kb * IE],
                                                in0=xg[:, xo:xo + kb * IE],
                                                in1=ps_s[:, :kb * IE],
                                                op=mybir.AluOpType.mult)
                        Bsb = sbp.tile([P, KB * WW], bf16, tag="Bsb")
                        nc.vector.tensor_tensor(
                            out=Bsb[:, :kb * WW].rearrange("p (j f) -> p j f", f=WW),
                            in0=off1_sb[:, c0:c0 + kb].to_broadcast([P, kb, WW]),
                            in1=iota_rep[:, :kb * WW].rearrange("p (j f) -> p j f", f=WW),
                            op=mybir.AluOpType.is_equal)
                        for j in range(kb):
                            ch = c0 + j
                            w_abs = ch // C1
                            c = ch % C1
                            w_rel = w_abs - ws_lo
                            sub = (w_rel * WW) // 512
                            if sub not in agg_tiles:
                                agg_tiles[sub] = pagg.tile([IE, 512], f32, tag="agg", name="aggps")
                            co = (w_rel * WW) % 512
                            nc.tensor.matmul(
                                agg_tiles[sub][:, co:co + WW],
                                m_sb[:, j * IE:(j + 1) * IE],
                                Bsb[:, j * WW:(j + 1) * WW],
                                start=(c == 0), stop=(c == C1 - 1))
                    for sub, tl in agg_tiles.items():
                        sw = min(512, wid - sub * 512)
                        nc.vector.tensor_copy(
                            agg_g[:, sub * 512: sub * 512 + sw], tl[:, :sw])

                    # ---- h chain on this group
                    psD = pbig.tile([H, GW3 * P], f32, tag="pb")
                    mm512(psD[:, :wid], w["Wji"][:], xe[:, cols], wid)
                    xji = hch.tile([H, GW3 * P], bf16, tag="xji")
                    silu_ps(xji[:, :wid], psD[:, :wid], w["bji"][:])

                    psD = pbig.tile([H, GW3 * P], f32, tag="pb")
                    mm512(psD[:, :wid], w["Wup"][:], agg_g[:, :wid], wid)
                    hbuf = hch.tile([H, GW3 * P], bf16, tag="hbuf")
                    silu_ps(hbuf[:, :wid], psD[:, :wid], 0.0)
                    nc.vector.tensor_add(hbuf[:, :wid], hbuf[:, :wid], xji[:, :wid])

                    def res_layer(hin, r):
                        psX = pbig.tile([H, GW3 * P], f32, tag="pb")
                        mm512(psX[:, :wid], w["Wres"](r, 0), hin[:, :wid], wid)
                        tt = hch.tile([H, GW3 * P], bf16, tag="res_t")
                        silu_ps(tt[:, :wid], psX[:, :wid], w["bres"](r, 0))
                        psX = pbig.tile([H, GW3 * P], f32, tag="pb")
                        mm512(psX[:, :wid], w["Wres"](r, 1), tt[:, :wid], wid)
                        t2 = hch.tile([H, GW3 * P], bf16, tag="res_t2")
                        silu_ps(t2[:, :wid], psX[:, :wid], w["bres"](r, 1))
                        nc.vector.tensor_add(hin[:, :wid], hin[:, :wid], t2[:, :wid])

                    res_layer(hbuf, 0)
                    psD = pbig.tile([H, GW3 * P], f32, tag="pb")
                    mm512(psD[:, :wid], w["Wlin"][:], hbuf[:, :wid], wid)
                    t3 = hch.tile([H, GW3 * P], bf16, tag="t3")
                    silu_ps(t3[:, :wid], psD[:, :wid], w["blin"][:])
                    h2 = hch.tile([H, GW3 * P], bf16, tag="h2")
                    nc.vector.tensor_add(h2[:, :wid], t3[:, :wid], xe[:, cols])
                    res_layer(h2, 1)
                    res_layer(h2, 2)
                    nc.vector.tensor_copy(xe[:, cols], h2[:, :wid])

            # ---------------- main program
            out_block(0, first=True)
            for b in range(NB):
                w = load_block_weights(b)
                phase1(b, w)
                nc.gpsimd.collective_compute(
                    "AllGather", mybir.AluOpType.bypass,
                    replica_groups=[list(range(ncores))],
                    ins=[ag_in], outs=[ag_out])
                phase23(b, w)
                out_block(b + 1, first=False)

            nc.sync.dma_start(d_P[:], P_sb[:])

    nc.compile()
    return nc


# ---------------------------------------------------------------- entry
def run_kernel(inputs, ncores=8, sim=False, trace=False, prebuilt=None):
    if sim:
        os.environ.setdefault("JAX_PLATFORMS", "cpu")
    else:
        os.environ["JAX_PLATFORMS"] = "axon"
    import concourse.bass_utils as bass_utils
    from concourse import bacc

    x = np.asarray(inputs["x"], np.float32)
    rbf = np.asarray(inputs["rbf"], np.float32)
    sbf = np.asarray(inputs["sbf"], np.float32)
    dims, data = host_prep(x, rbf, sbf, inputs["idx_kj"], inputs["idx_ji"],
                           inputs["idx_i"], inputs["num_nodes"], ncores)
    ws = prep_weights(inputs)

    if prebuilt is None:
        nc = bacc.Bacc("TRN2", target_bir_lowering=False, debug=False,
                       enable_asserts=False, num_devices=ncores)
        build_kernel(nc, dims, sim_safe=sim)
    else:
        nc = prebuilt

    in_maps = []
    for c in range(ncores):
        m = {k: np.ascontiguousarray(v[c]) for k, v in data.items()}
        m.update(ws)
        in_maps.append(m)

    if sim:
        from concourse.bass_interp import MultiCoreSim
        simu = MultiCoreSim(nc, num_cores=ncores)
        for c in range(ncores):
            for k, v in in_maps[c].items():
                simu.cores[c].tensor(k)[:] = v
        simu.simulate()
        outs = [np.array(simu.cores[c].tensor("P_out")) for c in range(ncores)]
        res = None
    else:
        res = bass_utils.run_bass_kernel_spmd(
            nc, in_maps, core_ids=list(range(ncores)), trace=trace,
            stitch_traces=False)
        outs = [res.results[c]["P_out"] for c in range(ncores)]

    N = int(inputs["num_nodes"])
    Pfull = np.zeros((N, OC), np.float32)
    n_lo = dims["n_lo"]; nodes_c = dims["nodes_c"]
    for c in range(ncores):
        Pfull[n_lo[c]: n_lo[c] + nodes_c[c], 0] = outs[c].reshape(-1)[: nodes_c[c]]
    return Pfull, res, dims


def kernel(**inputs) -> np.ndarray:
    out, _, _ = run_kernel(inputs, ncores=8, sim=False, trace=False)
    return out
